# revision 12
# baseline (speedup 1.0000x reference)
"""nn_Loss_20212116095273: 0.99*smooth_l1_map + 0.01*direction_loss on 8 trn2 cores.

Pedestrian-axis sharding: each core takes 25000 peds padded to 25088 (pad rows
are constructed so their direction-loss contribution is ~0 and their map
columns are dropped on unshard). Direction loss uses the bounded atan2
reparameterization theta = pi/2 - g, g = w*P(|w|), w = dot/(|cross|+|dot|+eps),
summed on-device via fused accum_out, all-reduced across cores, and applied to
the map in a final ACT pass.
"""
import sys

sys.path.insert(0, "/opt/trn_rl_repo")

import numpy as np

import concourse.bass as bass
import concourse.tile as tile
from concourse import mybir
from concourse._compat import with_exitstack  # noqa: F401
from concourse.bass_utils import run_bass_kernel_spmd

AF = mybir.ActivationFunctionType
OP = mybir.AluOpType
f32 = mybir.dt.float32

# ---- patch the tail drain (walrus in this container rejects >4 waits/inst) --
_MAX_WAITS_PER_INST = 1


def _patched_drain_and_barrier(self, tick_clock, wait_clock):
    from concourse.tile import ScopedClock

    nc = self.nc
    probe = mybir.InstNoOp(name=nc.get_next_instruction_name(), ins=[], outs=[])
    probe.engine = mybir.EngineType.SP
    wait_clock.add_sem_waits(probe, ScopedClock({None: tick_clock.global_clock}))
    waits = list(probe.sync_info.on_wait)
    assert self.sems is not None
    num_to_handle = {h.num: h for h in self.sems.allocated().values()}
    for i in range(0, len(waits), _MAX_WAITS_PER_INST):
        chunk = waits[i : i + _MAX_WAITS_PER_INST]
        nop = nc.sync.nop(nofuse=True)
        for w in chunk:
            nop.wait_op(num_to_handle[w.id], w.wait_value, "sem-ge")
    nc.sync.drain()
    nc.all_engine_barrier()
    popped = nc._tile_sem_poison_stack.pop()
    assert popped is self._sem_poison
    nc.clear_and_free_semaphores(list(self.sems.allocated().values()))
    nc.all_engine_barrier()


tile.TileContext._drain_and_barrier = _patched_drain_and_barrier

# This walrus also rejects >1 semaphore wait on ANY instruction. Tile attaches
# multiple waits at join points; split them onto standalone NOPs inserted just
# before the instruction (same engine, in-order execution => equivalent).
import bass_rust as _bass_rust

_orig_lower = tile.TileContext._lower_ordered_insts


def _split_multiwait_lower(self, ordered):
    nc = self.nc
    for bbname, insts in ordered.items():
        out = []
        for inst in insts:
            si = inst.sync_info
            if si is not None and len(si.on_wait) > 1:
                waits = list(si.on_wait)
                for w in waits[:-1]:
                    nop = mybir.InstNoOp(
                        name=nc.get_next_instruction_name(), ins=[], outs=[]
                    )
                    nop.engine = inst.engine
                    nop.bass_nofuse = True
                    nop.sync_info = _bass_rust.SyncInfo(
                        on_wait=[w], on_update=[]
                    )
                    out.append(nop)
                inst.sync_info = _bass_rust.SyncInfo(
                    on_wait=[waits[-1]], on_update=list(si.on_update)
                )
            out.append(inst)
        insts[:] = out
    return _orig_lower(self, ordered)


tile.TileContext._lower_ordered_insts = _split_multiwait_lower
# -----------------------------------------------------------------------------


def _act_recip(nc, out_ap, in_ap):
    """ACT-table reciprocal. bass blocks AF.Reciprocal over accuracy concerns;
    at this kernel's ~1%% tolerance the table accuracy is more than enough."""
    eng = nc.scalar
    inputs = [
        eng.lower_ap(in_ap),
        mybir.ImmediateValue(dtype=f32, value=0.0),
        mybir.ImmediateValue(dtype=f32, value=1.0),
        mybir.ImmediateValue(dtype=f32, value=0.0),
    ]
    return eng.add_instruction(
        mybir.InstActivation(
            name=nc.get_next_instruction_name(),
            func=AF.Reciprocal,
            ins=inputs,
            outs=[eng.lower_ap(out_ap)],
        )
    )

T = 16
F_DIR = 15
P_FULL = 200_000
N_CORES = 8
P_CORE = P_FULL // N_CORES  # 25000
P_PAD = 25_088  # 128 * 196

# deg-4 minimax fit of arctan(x/(1-x))/x on [0,1]
C0, C1, C2, C3, C4 = 1.0103447, 0.68558774, 2.7885366, -4.7763572, 1.8590031
EPS = 1e-20
SQRT_HALF = float(np.sqrt(0.5))
MAP_SCALE = 0.99 / P_FULL
DIR_SCALE = 0.01 * 0.2 / (P_FULL * F_DIR)


def build_program(p_pad=P_PAD, chunk=1792, n_cores=N_CORES, use_collective=True, repeat=1):
    assert p_pad % chunk == 0 and chunk % 128 == 0
    g = chunk // 128  # peds per partition per frame per chunk
    n_chunks = p_pad // chunk
    gtot_per_part = p_pad // 128

    nc = bass.Bass(num_devices=n_cores)
    outputs = nc.dram_tensor("outputs", [T, p_pad, 4], f32, kind="ExternalInput")
    targets = nc.dram_tensor("targets", [T, p_pad, 8], f32, kind="ExternalInput")
    y = nc.dram_tensor("y", [T, p_pad], f32, kind="ExternalOutput")
    cc_in = nc.dram_tensor("cc_in", [1, 1], f32)
    cc_out = nc.dram_tensor("cc_out", [1, 1], f32, addr_space="Shared")

    n_dir_elems_all = F_DIR * p_pad * 5 * n_cores
    # bias = (pi/2 * N - G_all) * DIR_SCALE
    bias_const = float(np.pi / 2 * n_dir_elems_all * DIR_SCALE)

    with tile.TileContext(nc) as tc:
        with (
            tc.tile_pool(name="io", bufs=2) as io,
            tc.tile_pool(name="sl1", bufs=1) as sl1p,
            tc.tile_pool(name="stage_a", bufs=1) as sa,
            tc.tile_pool(name="prod", bufs=1) as pr,
            tc.tile_pool(name="corner", bufs=1) as co,
            tc.tile_pool(name="mapp", bufs=1) as mapp,
            tc.tile_pool(name="accp", bufs=1) as accp,
            tc.tile_pool(name="tailp", bufs=1) as tailp,
            tc.tile_pool(name="psum", bufs=2, space="PSUM") as psum,
        ):
          for _rep in range(repeat):
            map_t = mapp.tile([128, T, gtot_per_part], f32, tag="map_t")
            gacc = accp.tile([128, n_chunks], f32, tag="gacc")

            for k in range(n_chunks):
                p0 = k * chunk
                # ---- loads: frame-batched, partition-major SBUF layout ----
                out_t = io.tile([128, T, g, 4], f32, tag="out_t")
                src_o = outputs[:, p0 : p0 + chunk, :].rearrange(
                    "f (p g) c -> p f g c", p=128
                )
                nc.sync.dma_start(out_t[:], src_o)
                tgt_t = io.tile([128, T, g, 8], f32, tag="tgt_t")
                src_t = targets[:, p0 : p0 + chunk, :].rearrange(
                    "f (p g) c -> p f g c", p=128
                )
                nc.sync.dma_start(tgt_t[:], src_t)

                def o(c, t0=0, t1=F_DIR):
                    return out_t[:, t0:t1, :, c]

                def tg(c, t0, t1):
                    return tgt_t[:, t0:t1, :, c]

                # ---- smooth L1 over all 16 frames ----
                d = sl1p.tile([128, T, g, 4], f32, tag="d")
                nc.vector.tensor_tensor(d[:], out_t[:], tgt_t[:, :, :, 4:8], OP.subtract)
                ad = sl1p.tile([128, T, g, 4], f32, tag="ad")
                nc.scalar.activation(ad[:], d[:], AF.Abs)
                m = sl1p.tile([128, T, g, 4], f32, tag="m")
                nc.vector.tensor_scalar(m[:], ad[:], 1.0, None, OP.min)
                r = sl1p.tile([128, T, g, 4], f32, tag="r")
                nc.vector.tensor_scalar(r[:], ad[:], 1.0, -1.0, OP.max, OP.add)
                sq = sl1p.tile([128, T, g, 4], f32, tag="sq")
                nc.scalar.activation(sq[:], m[:], AF.Square, scale=SQRT_HALF)
                sl = sl1p.tile([128, T, g, 4], f32, tag="sl")
                nc.vector.tensor_tensor(sl[:], sq[:], r[:], OP.add)
                nc.vector.tensor_reduce(
                    map_t[:, :, k * g : (k + 1) * g],
                    sl[:],
                    axis=mybir.AxisListType.X,
                    op=OP.add,
                )

                # ---- stage A: deltas [128, F_DIR, g] ----
                A1 = sa.tile([128, T, g], f32, tag="A1")
                nc.vector.scalar_tensor_tensor(
                    A1[:], tg(2, 0, T), -0.5, tg(0, 0, T), OP.mult, OP.add
                )
                B1 = sa.tile([128, T, g], f32, tag="B1")
                nc.vector.scalar_tensor_tensor(
                    B1[:], tg(3, 0, T), -0.5, tg(1, 0, T), OP.mult, OP.add
                )
                A0 = sa.tile([128, F_DIR, g], f32, tag="A0")
                nc.vector.scalar_tensor_tensor(
                    A0[:], tg(0, 0, F_DIR), -0.5, A1[:, 0:F_DIR], OP.mult, OP.add
                )
                B0 = sa.tile([128, F_DIR, g], f32, tag="B0")
                nc.vector.scalar_tensor_tensor(
                    B0[:], tg(1, 0, F_DIR), -0.5, B1[:, 0:F_DIR], OP.mult, OP.add
                )

                u0 = sa.tile([128, F_DIR, g], f32, tag="u0")
                nc.vector.scalar_tensor_tensor(
                    u0[:], o(2), -0.5, o(0), OP.mult, OP.add
                )
                v0 = sa.tile([128, F_DIR, g], f32, tag="v0")
                nc.vector.scalar_tensor_tensor(
                    v0[:], o(3), -0.5, o(1), OP.mult, OP.add
                )

                dX1p = sa.tile([128, F_DIR, g], f32, tag="dX1p")
                nc.vector.scalar_tensor_tensor(
                    dX1p[:, 1:], tg(2, 1, F_DIR), 0.5, o(0, 1), OP.mult, OP.add
                )
                nc.vector.tensor_copy(dX1p[:, 0:1], o(0, 0, 1))
                dY1p = sa.tile([128, F_DIR, g], f32, tag="dY1p")
                nc.vector.scalar_tensor_tensor(
                    dY1p[:, 1:], tg(3, 1, F_DIR), 0.5, o(1, 1), OP.mult, OP.add
                )
                nc.vector.tensor_copy(dY1p[:, 0:1], o(1, 0, 1))
                dX0p = sa.tile([128, F_DIR, g], f32, tag="dX0p")
                nc.vector.scalar_tensor_tensor(
                    dX0p[:, 1:], tg(0, 1, F_DIR), 0.5, u0[:, 1:], OP.mult, OP.add
                )
                nc.vector.tensor_copy(dX0p[:, 0:1], u0[:, 0:1])
                dY0p = sa.tile([128, F_DIR, g], f32, tag="dY0p")
                nc.vector.scalar_tensor_tensor(
                    dY0p[:, 1:], tg(1, 1, F_DIR), 0.5, v0[:, 1:], OP.mult, OP.add
                )
                nc.vector.tensor_copy(dY0p[:, 0:1], v0[:, 0:1])

                dX1t = sa.tile([128, F_DIR, g], f32, tag="dX1t")
                nc.vector.tensor_tensor(
                    dX1t[:, 1:], tg(0, 2, T), A1[:, 1:F_DIR], OP.subtract
                )
                nc.vector.tensor_tensor(
                    dX1t[:, 0:1], tg(0, 1, 2), tg(0, 0, 1), OP.subtract
                )
                dY1t = sa.tile([128, F_DIR, g], f32, tag="dY1t")
                nc.vector.tensor_tensor(
                    dY1t[:, 1:], tg(1, 2, T), B1[:, 1:F_DIR], OP.subtract
                )
                nc.vector.tensor_tensor(
                    dY1t[:, 0:1], tg(1, 1, 2), tg(1, 0, 1), OP.subtract
                )
                dX0t = sa.tile([128, F_DIR, g], f32, tag="dX0t")
                nc.vector.tensor_tensor(
                    dX0t[:, 1:], A1[:, 2:T], A0[:, 1:], OP.subtract
                )
                nc.vector.tensor_tensor(
                    dX0t[:, 0:1], A1[:, 1:2], A1[:, 0:1], OP.subtract
                )
                dY0t = sa.tile([128, F_DIR, g], f32, tag="dY0t")
                nc.vector.tensor_tensor(
                    dY0t[:, 1:], B1[:, 2:T], B0[:, 1:], OP.subtract
                )
                nc.vector.tensor_tensor(
                    dY0t[:, 0:1], B1[:, 1:2], B1[:, 0:1], OP.subtract
                )

                Xsp = sa.tile([128, F_DIR, g], f32, tag="Xsp")
                nc.vector.tensor_tensor(Xsp[:], dX0p[:], dX1p[:], OP.add)
                Ysp = sa.tile([128, F_DIR, g], f32, tag="Ysp")
                nc.vector.tensor_tensor(Ysp[:], dY0p[:], dY1p[:], OP.add)
                Xst = sa.tile([128, F_DIR, g], f32, tag="Xst")
                nc.vector.tensor_tensor(Xst[:], dX0t[:], dX1t[:], OP.add)
                Yst = sa.tile([128, F_DIR, g], f32, tag="Yst")
                nc.vector.tensor_tensor(Yst[:], dY0t[:], dY1t[:], OP.add)

                # ---- stage B: dots & crosses for 5 corners ----
                # corner order: 0:(X0,Y0) 1:(X0,Y1) 2:(X1,Y0) 3:(X1,Y1) 4:center
                dots_f = pr.tile([128, 5 * F_DIR * g], f32, tag="dots")
                crosses_f = pr.tile([128, 5 * F_DIR * g], f32, tag="crosses")
                dots = dots_f[:].rearrange("p (a b c) -> p a b c", b=F_DIR, c=g)
                crosses = crosses_f[:].rearrange("p (a b c) -> p a b c", b=F_DIR, c=g)

                def prod(name, a, b):
                    t = pr.tile([128, F_DIR, g], f32, tag=name)
                    nc.vector.tensor_tensor(t[:], a[:], b[:], OP.mult)
                    return t

                XX0 = prod("XX0", dX0p, dX0t)
                XX1 = prod("XX1", dX1p, dX1t)
                YY0 = prod("YY0", dY0p, dY0t)
                YY1 = prod("YY1", dY1p, dY1t)
                XXc = prod("XXc", Xsp, Xst)
                YYc = prod("YYc", Ysp, Yst)
                for ci, (xx, yy) in enumerate(
                    [(XX0, YY0), (XX0, YY1), (XX1, YY0), (XX1, YY1), (XXc, YYc)]
                ):
                    nc.vector.tensor_tensor(dots[:, ci], xx[:], yy[:], OP.add)

                XY00 = prod("XY00", dX0p, dY0t)
                XY01 = prod("XY01", dX0p, dY1t)
                XY10 = prod("XY10", dX1p, dY0t)
                XY11 = prod("XY11", dX1p, dY1t)
                XYc = prod("XYc", Xsp, Yst)
                YX00 = prod("YX00", dY0p, dX0t)
                YX01 = prod("YX01", dY1p, dX0t)
                YX10 = prod("YX10", dY0p, dX1t)
                YX11 = prod("YX11", dY1p, dX1t)
                YXc = prod("YXc", Ysp, Xst)
                for ci, (xy, yx) in enumerate(
                    [(XY00, YX00), (XY01, YX01), (XY10, YX10), (XY11, YX11), (XYc, YXc)]
                ):
                    nc.vector.tensor_tensor(crosses[:, ci], xy[:], yx[:], OP.subtract)

                # ---- corner chain on [128, 5*F_DIR*g] ----
                NC = 5 * F_DIR * g
                cu = co.tile([128, NC], f32, tag="cu")
                nc.scalar.activation(cu[:], crosses_f[:], AF.Abs)
                cva = co.tile([128, NC], f32, tag="cva")
                nc.scalar.activation(cva[:], dots_f[:], AF.Abs)
                den = co.tile([128, NC], f32, tag="den")
                nc.vector.scalar_tensor_tensor(
                    den[:], cu[:], EPS, cva[:], OP.add, OP.add
                )
                rden = co.tile([128, NC], f32, tag="rden")
                _act_recip(nc, rden[:], den[:])
                wt = co.tile([128, NC], f32, tag="wt")
                nc.vector.tensor_tensor(wt[:], dots_f[:], rden[:], OP.mult)
                xw = co.tile([128, NC], f32, tag="xw")
                nc.scalar.activation(xw[:], wt[:], AF.Abs)
                p = co.tile([128, NC], f32, tag="p")
                nc.vector.tensor_scalar(p[:], xw[:], C4, C3, OP.mult, OP.add)
                nc.vector.tensor_tensor(p[:], p[:], xw[:], OP.mult)
                nc.vector.tensor_scalar(p[:], p[:], C2, 0.0, OP.add, OP.add)
                nc.vector.tensor_tensor(p[:], p[:], xw[:], OP.mult)
                nc.vector.tensor_scalar(p[:], p[:], C1, 0.0, OP.add, OP.add)
                nc.vector.tensor_tensor(p[:], p[:], xw[:], OP.mult)
                nc.vector.tensor_scalar(p[:], p[:], C0, 0.0, OP.add, OP.add)
                gt = co.tile([128, NC], f32, tag="gt")
                nc.vector.scalar_tensor_tensor(
                    gt[:], p[:], 0.0, wt[:], OP.add, OP.mult,
                    accum_out=gacc[:, k : k + 1],
                )

            # ---- tail: reduce, all-reduce, bias, final map pass ----
            gtot = tailp.tile([128, 1], f32)
            nc.vector.tensor_reduce(
                gtot[:], gacc[:], axis=mybir.AxisListType.X, op=OP.add
            )
            ones_col = tailp.tile([128, 1], f32)
            nc.vector.memset(ones_col[:], 1.0)
            gsum = psum.tile([1, 1], f32)
            nc.tensor.matmul(gsum[:], gtot[:], ones_col[:], start=True, stop=True)
            gsum_s = tailp.tile([1, 1], f32)
            nc.vector.tensor_copy(gsum_s[:], gsum[:])
            nc.sync.dma_start(cc_in[:], gsum_s[:])
            gall = tailp.tile([1, 1], f32)
            if use_collective:
                nc.gpsimd.collective_compute(
                    "AllReduce",
                    OP.add,
                    replica_groups=[list(range(n_cores))],
                    ins=[cc_in[:]],
                    outs=[cc_out[:]],
                )
                nc.sync.dma_start(gall[:], cc_out[:])
            else:
                nc.sync.dma_start(gall[:], cc_in[:])
            bscal = tailp.tile([1, 1], f32)
            nc.vector.tensor_scalar(
                bscal[:], gall[:], -DIR_SCALE, bias_const, OP.mult, OP.add
            )
            ones_row = tailp.tile([1, 128], f32)
            nc.vector.memset(ones_row[:], 1.0)
            bias_ps = psum.tile([128, 1], f32)
            nc.tensor.matmul(bias_ps[:], ones_row[:], bscal[:], start=True, stop=True)
            bias_s = tailp.tile([128, 1], f32)
            nc.vector.tensor_copy(bias_s[:], bias_ps[:])

            for k in range(n_chunks):
                out_f = mapp.tile([128, T, g], f32, tag="out_f", bufs=2)
                nc.scalar.activation(
                    out_f[:],
                    map_t[:, :, k * g : (k + 1) * g],
                    AF.Identity,
                    bias=bias_s[:],
                    scale=MAP_SCALE,
                )
                dst_y = y[:, k * chunk : (k + 1) * chunk].rearrange(
                    "f (p g) -> p f g", p=128
                )
                nc.sync.dma_start(dst_y, out_f[:])

    return nc


def _pad_block(n_pad):
    """Pad rows with exactly-zero direction-loss contribution (pred==true dirs)."""
    tgt = np.zeros((T, n_pad, 8), dtype=np.float32)
    for t in range(T):
        tgt[t, :, 0] = t + 1.0
        tgt[t, :, 1] = t + 1.0
        tgt[t, :, 2] = 1.0
        tgt[t, :, 3] = 1.0
    out = np.zeros((T, n_pad, 4), dtype=np.float32)
    out[:, :, 0] = 1.0
    out[:, :, 1] = 1.0
    return out, tgt


_NC_CACHE = {}


def _get_program():
    if "nc" not in _NC_CACHE:
        _NC_CACHE["nc"] = build_program()
    return _NC_CACHE["nc"]


def _make_in_maps(outputs, targets):
    pad_o, pad_t = _pad_block(P_PAD - P_CORE)
    in_maps = []
    for c in range(N_CORES):
        sl = slice(c * P_CORE, (c + 1) * P_CORE)
        o_s = np.ascontiguousarray(
            np.concatenate([outputs[:, sl, :], pad_o], axis=1), dtype=np.float32
        )
        t_s = np.ascontiguousarray(
            np.concatenate([targets[:, sl, :], pad_t], axis=1), dtype=np.float32
        )
        in_maps.append({"outputs": o_s, "targets": t_s})
    return in_maps


def run_on_hw(outputs, targets, **kwargs):
    nc = _get_program()
    in_maps = _make_in_maps(outputs, targets)
    res = run_bass_kernel_spmd(nc, in_maps, core_ids=list(range(N_CORES)), **kwargs)
    full = np.empty((T, P_FULL), dtype=np.float32)
    for c in range(N_CORES):
        full[:, c * P_CORE : (c + 1) * P_CORE] = res.results[c]["y"][:, :P_CORE]
    return full, res


def kernel(outputs: np.ndarray, targets: np.ndarray) -> np.ndarray:
    outputs = np.asarray(outputs, dtype=np.float32)
    targets = np.asarray(targets, dtype=np.float32)
    full, _ = run_on_hw(outputs, targets)
    return full
